# revision 31
# baseline (speedup 1.0000x reference)
"""Multi-head attention (B=2, N=2048, C=1024, H=16, D=64) on 8 TRN2 cores.

Sharding: tensor-parallel over heads — 2 heads per core. Each core computes
Q/K/V projections for its 2 heads, attention, and a partial output
projection (its heads' slice of Wo). Host sums the 8 partial outputs + bo.

Per-core dataflow (all matmul inputs bf16, PSUM accumulation fp32):
  xT [1024, 4096] (x transposed on host, replicated to all cores)
  QT/KT = W.T @ x.T   -> [128 (2 heads x 64), 4096]  (lhsT=W chunk, rhs=xT)
  VT likewise, then PE-transposed into v_aug [keys, 65] per head
  (65th column = ones -> softmax denominator comes out of the ctx matmul)
  S^T = K @ Q.T  -> [keys, q] in PSUM; exp on ScalarE -> bf16 SBUF
  ctx^T_aug [65, q] = v_aug.T @ expS^T  (row 64 = denominator)
  normalize: recip(row 64), gpsimd partition_broadcast, DVE multiply
  out_partial [4096, 1024] bf16 = ctx^T.T @ Wo_slice  (summed on host in f32)

Schedule: the kernel is ScalarE-bound (128 exp instructions of
[128,1024], ~1.11us each).  The emission order keeps the exp stream
dense from ~10us on: only the wk0+wq0 chains run before attention
starts; every other projection / V-transpose / output-projection is
woven into per-kc fill slots of the attention loop.  Scores for kc run
2-head-packed on the PE (row tiling via base_partition 0/64); ctx lags
scores by 2 kc so the PE never blocks on the exp.  HAM is warmed with
REGULAR matmuls during the initial DMA (transpose-mode does not count
as PE-busy for HAM).  The 1/sqrt(D) scale is folded into Wq/bq (0.125).
"""

import numpy as np
import ml_dtypes

import concourse.bass as bass
from concourse import bacc
import concourse.tile as tile
from concourse import mybir, library_config
from concourse.bass_utils import run_bass_kernel_spmd

BF16 = mybir.dt.bfloat16
F32 = mybir.dt.float32

B, N, C = 2, 2048, 1024
H, D = 16, 64
T = B * N              # 4096 tokens
HPC = H // 8           # heads per core = 2
DPC = HPC * D          # head dims per core = 128

KCH = C // 128         # 8 contraction chunks for projections
NCH = T // 512         # 8 token chunks of 512
KT16 = N // 128        # 16 key tiles per batch
CCH = T // 1024        # 4 token blocks of 1024 for the xT DMA tiles


def build_core_program(nc):
    xT = nc.dram_tensor("xT", [C, T], BF16, kind="ExternalInput").ap()
    wq = nc.dram_tensor("wq", [C, DPC], BF16, kind="ExternalInput").ap()
    wk = nc.dram_tensor("wk", [C, DPC], BF16, kind="ExternalInput").ap()
    wv = nc.dram_tensor("wv", [C, DPC], BF16, kind="ExternalInput").ap()
    wo = nc.dram_tensor("wo", [DPC, C], BF16, kind="ExternalInput").ap()
    bqkv = nc.dram_tensor("bqkv", [DPC, 3], F32, kind="ExternalInput").ap()
    iden = nc.dram_tensor("iden", [128, 128], BF16, kind="ExternalInput").ap()
    out = nc.dram_tensor("out", [T, C], BF16, kind="ExternalOutput").ap()

    with tile.TileContext(nc) as tc:
        with tc.tile_pool(name="singles", bufs=1) as singles:
            nc.gpsimd.load_library(library_config.proxy)

            # DMA layout: each transfer has ~650ns of fixed overhead, so
            # xT moves as 16 half-MB tiles, not more smaller ones, split
            # across BOTH hwdge queues (sync + ACT) by k parity.  The
            # first-needed weights (iden/bqkv/wk/wq) go ahead of the xT
            # stream on the ACT queue; wv/wo follow the first xT block.
            id_sb = singles.tile([128, 128], BF16, tag="iden")
            nc.scalar.dma_start(out=id_sb, in_=iden)
            bqkv_sb = singles.tile([DPC, 3], F32, tag="bqkv")
            nc.scalar.dma_start(out=bqkv_sb, in_=bqkv)
            b_sb = {"bq": bqkv_sb[:, 0:1], "bk": bqkv_sb[:, 1:2],
                    "bv": bqkv_sb[:, 2:3]}
            # Both hwdge queues share ONE DMA engine/AXI port at ~300GB/s
            # per core (64MB of 8-core-replicated xT ≈ 23us chip-wide), so
            # ordering on a single queue IS the arrival order.  Strict
            # need order on sync: tokens 0-1023 of xT (covers every
            # chain chunk 0 touches in its first half), wk, wq, wv,
            # tokens 1024-2047, wo, then batch 1.  The ACT queue carries
            # only the two tiny early tensors (its transfers hold the ACT
            # engine, which is harmless pre-attention).
            w_sb = {}
            wdefs = {"wk": wk, "wq": wq, "wv": wv}

            xTr = xT.rearrange("(k p) t -> p k t", p=128)
            xts = singles.tile([128, KCH, 1024], BF16, tag="xts")
            nc.sync.dma_start(out=xts, in_=xTr[:, :, 0:1024])
            for nm in ("wk", "wq", "wv"):
                t = singles.tile([128, KCH, DPC], BF16, tag=f"w{nm}",
                                 name=f"w{nm}")
                nc.sync.dma_start(
                    out=t, in_=wdefs[nm].rearrange("(k p) j -> p k j", p=128))
                w_sb[nm] = [t[:, k, :] for k in range(KCH)]
            xtm = singles.tile([128, KCH, 1024], BF16, tag="xtm")
            nc.sync.dma_start(out=xtm, in_=xTr[:, :, 1024:2048])
            wo_sb = singles.tile([DPC, C], BF16, tag="wo")
            nc.sync.dma_start(out=wo_sb, in_=wo)
            xtb = [singles.tile([128, 2048], BF16, tag=f"xtb{k}",
                                name=f"xtb{k}") for k in range(KCH)]
            for k in range(KCH):
                nc.sync.dma_start(
                    out=xtb[k], in_=xT[k * 128:(k + 1) * 128, 2048:4096])

            def xt_rhs(k, nch):
                if nch < 2:
                    return xts[:, k, nch * 512:nch * 512 + 512]
                if nch < 4:
                    off = (nch - 2) * 512
                    return xtm[:, k, off:off + 512]
                off = (nch - 4) * 512
                return xtb[k][:, off:off + 512]

            QT = singles.tile([128, T], BF16, tag="QT")
            KTt = singles.tile([128, T], BF16, tag="KT")
            VT = singles.tile([128, T], BF16, tag="VT")
            ctxTn = singles.tile([128, T], BF16, tag="ctxTn")
            vaug = [[singles.tile([128, KT16, D + 1], BF16,
                                  tag=f"vaug{b}{h}", name=f"vaug{b}{h}")
                     for h in range(HPC)] for b in range(B)]
            for b in range(B):
                for h in range(HPC):
                    nc.vector.memset(vaug[b][h], 1.0)


            with tc.tile_pool(name="psP", bufs=1, space="PSUM") as psP, \
                    tc.tile_pool(name="psO", bufs=1, space="PSUM") as psO, \
                    tc.tile_pool(name="psS", bufs=2, space="PSUM") as psS, \
                    tc.tile_pool(name="psC", bufs=1, space="PSUM") as psC, \
                    tc.tile_pool(name="esb", bufs=6) as esb, \
                    tc.tile_pool(name="nrm", bufs=4) as nrm, \
                    tc.tile_pool(name="csb", bufs=3) as csb, \
                    tc.tile_pool(name="osb", bufs=3) as osb:

                # REGULAR matmuls (not transposes: transpose-mode doesn't
                # count as PE-busy for HAM) to warm the clock gate to
                # K=8/8.  id_sb as both operands: no data deps beyond the
                # tiny iden DMA.  Interleaved between startup chain parts
                # (below) so the PE stays dense through the DMA-paced
                # region and the first chains run at full clock.
                warm_ctr = [0]

                def emit_warm(n):
                    # psS only: its 2 bufs alternate banks so junk MMs
                    # never serialize against the real psP/psO users.
                    for _ in range(n):
                        warm_ctr[0] += 1
                        ptw = psS.tile([128, 128], F32, tag="s",
                                       name="warm")
                        nc.tensor.matmul(out=ptw, lhsT=id_sb, rhs=id_sb,
                                         start=True, stop=True)

                # ---- projection chains, split into 2-matmul parts ----
                def chain_items(nm, dstT, nch, act_bias=False):
                    """Return 4 fill items (2 matmuls each); the last also
                    applies the bias.  V-transposes are separate items."""
                    ps_box = []

                    def part(p):
                        def emit():
                            if p == 0:
                                ps_box.append(
                                    psP.tile([128, 512], F32, tag="pj",
                                             name="pj"))
                            ps = ps_box[0]
                            for k in range(2 * p, 2 * p + 2):
                                nc.tensor.matmul(
                                    out=ps, lhsT=w_sb[nm][k],
                                    rhs=xt_rhs(k, nch),
                                    start=(k == 0), stop=(k == KCH - 1))
                            if p == 3:
                                dst = dstT[:, nch * 512:(nch + 1) * 512]
                                if act_bias:
                                    nc.scalar.activation(
                                        out=dst, in_=ps,
                                        func=mybir.ActivationFunctionType.Identity,
                                        bias=b_sb["b" + nm[1]], scale=1.0)
                                else:
                                    nc.vector.tensor_scalar_add(
                                        out=dst, in0=ps,
                                        scalar1=b_sb["b" + nm[1]])
                        return emit
                    return [part(p) for p in range(4)]

                def vtrans_items(b, nch):
                    """2 items, each transposing 2 of the 4 just-projected
                    128-token V tiles of (b, nch) into v_aug."""
                    def pair(j):
                        def emit():
                            for t16 in range(nch * 4 + 2 * j,
                                             nch * 4 + 2 * j + 2):
                                bt = t16 % KT16
                                pt = psO.tile([128, 128], BF16, tag="po",
                                              name="pt")
                                base = (b * N + bt * 128)
                                nc.tensor.transpose(
                                    pt, VT[:, base:base + 128], id_sb)
                                nc.vector.tensor_copy(
                                    out=vaug[b][0][:, bt, 0:D],
                                    in_=pt[:, 0:D])
                                nc.vector.tensor_copy(
                                    out=vaug[b][1][:, bt, 0:D],
                                    in_=pt[:, D:2 * D])
                        return emit
                    return [pair(0), pair(1)]

                def outproj_items(qch, last=False):
                    """8 items: one [128tok,512] out-proj matmul + copy
                    each; DMA after every 4 (one ot tile per nch2)."""
                    q0 = qch * 512
                    ot_box = {}

                    def item(j):
                        nch2, t4 = divmod(j, 4)

                        def emit():
                            if t4 == 0:
                                ot_box[nch2] = osb.tile(
                                    [128, 4, 512], BF16, tag="ot", name="ot")
                            ot = ot_box[nch2]
                            po = psO.tile([128, 512], F32, tag="po",
                                          name="po")
                            tok = q0 + t4 * 128
                            nc.tensor.matmul(
                                out=po, lhsT=ctxTn[:, tok:tok + 128],
                                rhs=wo_sb[:, nch2 * 512:(nch2 + 1) * 512],
                                start=True, stop=True)
                            nc.vector.tensor_copy(ot[:, t4, :], po)
                            if t4 == 3:
                                nc.sync.dma_start(
                                    out=out[q0:q0 + 512,
                                            nch2 * 512:(nch2 + 1) * 512]
                                    .rearrange("(t p) c -> p t c", p=128),
                                    in_=ot)
                        return emit
                    return [item(j) for j in range(8)]

                # ---- attention pieces ----
                def emit_scores(b, qch, kc):
                    # high_priority: at chunk boundaries the first scores
                    # matmul stalls ~1us on its psS bank; without the
                    # priority boost the scheduler slots fill matmuls
                    # between the two scores MMs of the pair, delaying the
                    # exp (the critical engine) by ~2us per boundary.
                    q0 = b * N + qch * 512
                    k0 = b * N + kc * 128
                    with tc.high_priority(offset=8):
                        pS = psS.tile([128, 1024], F32, tag="s", name="s")
                        for h in range(HPC):
                            nc.tensor.matmul(
                                out=pS[:, h * 512:(h + 1) * 512],
                                lhsT=KTt[h * D:(h + 1) * D, k0:k0 + 128],
                                rhs=QT[h * D:(h + 1) * D, q0:q0 + 512],
                                start=True, stop=True)
                        eS = esb.tile([128, 1024], BF16, tag="e", name="e")
                        nc.scalar.activation(
                            eS, pS, mybir.ActivationFunctionType.Exp)
                    return eS

                def emit_ctx(b, kc, eS, box):
                    for h in range(HPC):
                        nc.tensor.matmul(
                            out=box["t"][h], lhsT=vaug[b][h][:, kc, :],
                            rhs=eS[:, h * 512:(h + 1) * 512],
                            start=(kc == 0), stop=(kc == KT16 - 1))

                def emit_normalize(b, qch, ctx, last=False):
                    """ctx rows 0..63 / row 64 -> ctxTn (bf16).  Returns
                    (bcs, ctxss) when last=True for the per-qtile tail.
                    The cheap dn/recip ops go first on the DVE so both
                    gpsimd broadcasts start early; the heavy ctxs copies
                    overlap them."""
                    q0 = b * N + qch * 512
                    bcs, ctxss, rcs = [], [], []
                    for h in range(HPC):
                        dn = nrm.tile([1, 512], F32, tag=f"dn{h}",
                                      name=f"dn{h}")
                        nc.vector.tensor_copy(dn, ctx[h][D:D + 1, :])
                        rc = nrm.tile([1, 512], F32, tag=f"rc{h}",
                                      name=f"rc{h}")
                        nc.vector.reciprocal_approx_fast(rc, dn)
                        rcs.append(rc)
                    for h in range(HPC):
                        bc = nrm.tile([D, 512], F32, tag=f"bc{h}",
                                      name=f"bc{h}")
                        nc.gpsimd.partition_broadcast(bc, rcs[h])
                        bcs.append(bc)
                        ctxs = csb.tile([D, 512], F32, tag=f"ctxs{h}",
                                        name=f"ctxs{h}")
                        nc.vector.tensor_copy(ctxs, ctx[h][0:D, :])
                        ctxss.append(ctxs)
                    if last:
                        return bcs, ctxss
                    for h in range(HPC):
                        eng = nc.gpsimd if h == 0 else nc.vector
                        eng.tensor_mul(
                            out=ctxTn[h * D:(h + 1) * D, q0:q0 + 512],
                            in0=ctxss[h], in1=bcs[h])
                    return None

                # ---- chunk driver ----
                # chunk index ch = 0..7 -> (b, qch)
                def interleave(a, b):
                    o, i = [], 0
                    for x in a:
                        o.append(x)
                        if i < len(b):
                            o.append(b[i])
                            i += 1
                    o.extend(b[i:])
                    return o

                def fills_for(ch):
                    if ch == 0:
                        return []  # chunk 0 has an explicit slot plan
                    f = []
                    if ch <= 6:
                        # wq for the NEXT chunk, ACT bias: its write of QT
                        # must not queue behind the DVE backlog or the
                        # next chunk's first scores (and exp) stall.
                        f += chain_items("wq", QT, ch + 1, act_bias=True)
                    if ch in (1, 2, 3):
                        nch = ch + 3  # wk4/wv4 .. wk6/wv6
                        f += chain_items("wk", KTt, nch)
                        f += chain_items("wv", VT, nch)
                        f += vtrans_items(1, nch - 4)
                    if ch == 4:
                        f = (chain_items("wk", KTt, 7)
                             + chain_items("wv", VT, 7)
                             + vtrans_items(1, 3) + f)
                    # spread the out-proj items (each briefly serializes
                    # on the psO bank against its copy) between the chain
                    # items, and cap them at one per slot in the driver.
                    f = interleave([(False, x) for x in f],
                                   [(True, x) for x in outproj_items(ch - 1)])
                    return f

                # chunk 0 explicit slot plan: 3 items per kc slot
                c0_items = [(False, x) for x in (
                    chain_items("wv", VT, 0) + vtrans_items(0, 0)
                    + chain_items("wk", KTt, 1)
                    + chain_items("wv", VT, 1) + vtrans_items(0, 1)
                    + chain_items("wk", KTt, 2)
                    + chain_items("wv", VT, 2) + vtrans_items(0, 2)
                    + chain_items("wk", KTt, 3)
                    + chain_items("wv", VT, 3) + vtrans_items(0, 3)
                    + chain_items("wq", QT, 1, act_bias=True)
                )]

                # startup: only wk0 + wq0 before the exp stream begins;
                # warmup matmuls pad the DMA-paced stretches so HAM
                # reaches (and holds) K=8/8 before the chains run.
                emit_warm(8)
                for it in chain_items("wk", KTt, 0, act_bias=True):
                    emit_warm(4)
                    it()
                for it in chain_items("wq", QT, 0, act_bias=True):
                    emit_warm(2)
                    it()

                pend_ctx = []   # [(b, kc, eS, ctx)] lag-2 queue
                prev_norm = None

                for ch in range(8):
                    b, qch = divmod(ch, 4)
                    # psC tiles are allocated at kc==1, strictly AFTER the
                    # previous chunk's deferred ctx + normalize reads are
                    # emitted — allocating earlier would let this chunk's
                    # ctx matmuls race the previous normalize's PSUM reads.
                    box = {}
                    fills = c0_items if ch == 0 else fills_for(ch)
                    fi = 0
                    per_slot = 3 if ch == 0 else 2
                    for kc in range(KT16):
                        eS = emit_scores(b, qch, kc)
                        if pend_ctx and "t" in pend_ctx[0][3]:
                            emit_ctx(*pend_ctx.pop(0))
                        pend_ctx.append((b, kc, eS, box))
                        if kc == 1:
                            if prev_norm is not None:
                                # previous chunk's ctx is now fully emitted
                                prev_norm()
                                prev_norm = None
                            box["t"] = [
                                psC.tile([D + 1, 512], F32, tag=f"ctx{h}",
                                         name=f"ctx{h}")
                                for h in range(HPC)]
                        if prev_norm is not None:
                            # fills may include out-proj reads of ctxTn
                            # regions the pending normalize writes; popping
                            # them first would invert the dependency and
                            # read stale SBUF.
                            continue
                        took_op = False
                        for _ in range(per_slot):
                            if fi >= len(fills):
                                break
                            is_op, fn = fills[fi]
                            if is_op and took_op:
                                break  # max one psO out-proj item per slot
                            fn()
                            fi += 1
                            took_op = took_op or is_op
                    while fi < len(fills):
                        fills[fi][1]()
                        fi += 1

                    if ch < 7:
                        # leave the last 2 ctx for the next chunk's first
                        # slots (uniform lag across the boundary)
                        def mk_norm(b=b, qch=qch, ctx=box["t"]):
                            return lambda: emit_normalize(b, qch, ctx)
                        prev_norm = mk_norm()
                    else:
                        # tail: drain ctx, then a fully per-qtile pipeline
                        # (normalize 128 cols -> out-proj -> DMA) so the
                        # first out-proj matmul starts ~2us after the last
                        # exp instead of waiting for the 512-wide
                        # normalize chain.  Junk warm matmuls keep HAM at
                        # K=8/8 through the DVE/gpsimd-paced stretches.
                        while pend_ctx:
                            emit_ctx(*pend_ctx.pop(0))
                        emit_warm(4)
                        ctx_t = box["t"]
                        q0 = b * N + qch * 512
                        for t4 in range(4):
                            lo, hi = t4 * 128, (t4 + 1) * 128
                            bcs, ctxss = [], []
                            for h in range(HPC):
                                dn = nrm.tile([1, 128], F32, tag=f"dn{h}",
                                              name=f"dn{h}")
                                nc.vector.tensor_copy(
                                    dn, ctx_t[h][D:D + 1, lo:hi])
                                rc = nrm.tile([1, 128], F32, tag=f"rc{h}",
                                              name=f"rc{h}")
                                nc.vector.reciprocal_approx_fast(rc, dn)
                                bc = nrm.tile([D, 128], F32, tag=f"bc{h}",
                                              name=f"bc{h}")
                                nc.gpsimd.partition_broadcast(bc, rc)
                                bcs.append(bc)
                                ctxs = csb.tile([D, 128], F32,
                                                tag=f"ctxs{h}",
                                                name=f"ctxs{h}")
                                nc.vector.tensor_copy(
                                    ctxs, ctx_t[h][0:D, lo:hi])
                                ctxss.append(ctxs)
                            for h in range(HPC):
                                eng = nc.gpsimd if h == 0 else nc.vector
                                eng.tensor_mul(
                                    out=ctxTn[h * D:(h + 1) * D,
                                              q0 + lo:q0 + hi],
                                    in0=ctxss[h], in1=bcs[h])
                            emit_warm(2)
                            # both 512-wide halves into one 2-bank psS
                            # tile: one CAST + one DMA per token tile
                            # instead of two of each.
                            tok = q0 + t4 * 128
                            po = psS.tile([128, 1024], F32, tag="s",
                                          name="pol")
                            for nch2 in range(2):
                                nc.tensor.matmul(
                                    out=po[:, nch2 * 512:(nch2 + 1) * 512],
                                    lhsT=ctxTn[:, tok:tok + 128],
                                    rhs=wo_sb[:, nch2 * 512:(nch2 + 1) * 512],
                                    start=True, stop=True)
                            otl = osb.tile([128, 1024], BF16, tag="otl",
                                           name="otl", bufs=3)
                            nc.vector.tensor_copy(otl, po)
                            nc.sync.dma_start(
                                out=out[tok:tok + 128, :], in_=otl)
    return nc


_NC_CACHE = None


def _get_nc():
    global _NC_CACHE
    if _NC_CACHE is None:
        nc = bacc.Bacc("TRN2", target_bir_lowering=False)
        build_core_program(nc)
        nc.finalize()
        _NC_CACHE = nc
    return _NC_CACHE


def make_in_maps(x, Wq, bq, Wk, bk, Wv, bv, Wo):
    bf = ml_dtypes.bfloat16
    x = np.asarray(x, np.float32).reshape(T, C)
    xT_bf = np.ascontiguousarray(x.T).astype(bf)
    iden = np.eye(128, dtype=bf)
    Wq = np.asarray(Wq, np.float32)
    Wk = np.asarray(Wk, np.float32)
    Wv = np.asarray(Wv, np.float32)
    Wo = np.asarray(Wo, np.float32)
    bq = np.asarray(bq, np.float32)
    bk = np.asarray(bk, np.float32)
    bv = np.asarray(bv, np.float32)
    in_maps = []
    for cidx in range(8):
        hs = slice(cidx * DPC, (cidx + 1) * DPC)
        in_maps.append(dict(
            xT=xT_bf,
            wq=np.ascontiguousarray(Wq[:, hs] * 0.125).astype(bf),
            wk=np.ascontiguousarray(Wk[:, hs]).astype(bf),
            wv=np.ascontiguousarray(Wv[:, hs]).astype(bf),
            wo=np.ascontiguousarray(Wo[hs, :]).astype(bf),
            bqkv=np.stack([bq[hs] * 0.125, bk[hs], bv[hs]],
                          axis=1).astype(np.float32),
            iden=iden,
        ))
    return in_maps


def kernel(x, Wq, bq, Wk, bk, Wv, bv, Wo, bo, _trace=False, _trace_kwargs=None):
    in_maps = make_in_maps(x, Wq, bq, Wk, bk, Wv, bv, Wo)
    nc = _get_nc()
    res = run_bass_kernel_spmd(
        nc, in_maps, core_ids=list(range(8)),
        trace=_trace, **(_trace_kwargs or {}))
    acc = res.results[0]["out"].astype(np.float32)
    for cidx in range(1, 8):
        acc += res.results[cidx]["out"].astype(np.float32)
    acc += np.asarray(bo, np.float32)[None, :]
    out = acc.reshape(B, N, C)
    kernel.last_results = res
    return out


# revision 32
# speedup vs baseline: 1.0140x; 1.0140x over previous
"""Multi-head attention (B=2, N=2048, C=1024, H=16, D=64) on 8 TRN2 cores.

Sharding: tensor-parallel over heads — 2 heads per core. Each core computes
Q/K/V projections for its 2 heads, attention, and a partial output
projection (its heads' slice of Wo). Host sums the 8 partial outputs + bo.

Per-core dataflow (all matmul inputs bf16, PSUM accumulation fp32):
  xT [1024, 4096] (x transposed on host, replicated to all cores)
  QT/KT = W.T @ x.T   -> [128 (2 heads x 64), 4096]  (lhsT=W chunk, rhs=xT)
  VT likewise, then PE-transposed into v_aug [keys, 65] per head
  (65th column = ones -> softmax denominator comes out of the ctx matmul)
  S^T = K @ Q.T  -> [keys, q] in PSUM; exp on ScalarE -> bf16 SBUF
  ctx^T_aug [65, q] = v_aug.T @ expS^T  (row 64 = denominator)
  normalize: recip(row 64), gpsimd partition_broadcast, DVE multiply
  out_partial [4096, 1024] bf16 = ctx^T.T @ Wo_slice  (summed on host in f32)

Schedule: the kernel is ScalarE-bound (128 exp instructions of
[128,1024], ~1.11us each).  The emission order keeps the exp stream
dense from ~10us on: only the wk0+wq0 chains run before attention
starts; every other projection / V-transpose / output-projection is
woven into per-kc fill slots of the attention loop.  Scores for kc run
2-head-packed on the PE (row tiling via base_partition 0/64); ctx lags
scores by 2 kc so the PE never blocks on the exp.  HAM is warmed with
REGULAR matmuls during the initial DMA (transpose-mode does not count
as PE-busy for HAM).  The 1/sqrt(D) scale is folded into Wq/bq (0.125).
"""

import numpy as np
import ml_dtypes

import concourse.bass as bass
from concourse import bacc
import concourse.tile as tile
from concourse import mybir, library_config
from concourse.bass_utils import run_bass_kernel_spmd

BF16 = mybir.dt.bfloat16
F32 = mybir.dt.float32

B, N, C = 2, 2048, 1024
H, D = 16, 64
T = B * N              # 4096 tokens
HPC = H // 8           # heads per core = 2
DPC = HPC * D          # head dims per core = 128

KCH = C // 128         # 8 contraction chunks for projections
NCH = T // 512         # 8 token chunks of 512
KT16 = N // 128        # 16 key tiles per batch
CCH = T // 1024        # 4 token blocks of 1024 for the xT DMA tiles


def build_core_program(nc):
    xT = nc.dram_tensor("xT", [C, T], BF16, kind="ExternalInput").ap()
    wq = nc.dram_tensor("wq", [C, DPC], BF16, kind="ExternalInput").ap()
    wk = nc.dram_tensor("wk", [C, DPC], BF16, kind="ExternalInput").ap()
    wv = nc.dram_tensor("wv", [C, DPC], BF16, kind="ExternalInput").ap()
    wo = nc.dram_tensor("wo", [DPC, C], BF16, kind="ExternalInput").ap()
    bqkv = nc.dram_tensor("bqkv", [DPC, 3], F32, kind="ExternalInput").ap()
    iden = nc.dram_tensor("iden", [128, 128], BF16, kind="ExternalInput").ap()
    out = nc.dram_tensor("out", [T, C], BF16, kind="ExternalOutput").ap()

    with tile.TileContext(nc) as tc:
        with tc.tile_pool(name="singles", bufs=1) as singles:
            nc.gpsimd.load_library(library_config.proxy)

            # DMA layout: each transfer has ~650ns of fixed overhead, so
            # xT moves as 16 half-MB tiles, not more smaller ones, split
            # across BOTH hwdge queues (sync + ACT) by k parity.  The
            # first-needed weights (iden/bqkv/wk/wq) go ahead of the xT
            # stream on the ACT queue; wv/wo follow the first xT block.
            id_sb = singles.tile([128, 128], BF16, tag="iden")
            nc.scalar.dma_start(out=id_sb, in_=iden)
            bqkv_sb = singles.tile([DPC, 3], F32, tag="bqkv")
            nc.scalar.dma_start(out=bqkv_sb, in_=bqkv)
            b_sb = {"bq": bqkv_sb[:, 0:1], "bk": bqkv_sb[:, 1:2],
                    "bv": bqkv_sb[:, 2:3]}
            # Both hwdge queues share ONE DMA engine/AXI port at ~300GB/s
            # per core (64MB of 8-core-replicated xT ≈ 23us chip-wide), so
            # ordering on a single queue IS the arrival order.  Strict
            # need order on sync: tokens 0-1023 of xT (covers every
            # chain chunk 0 touches in its first half), wk, wq, wv,
            # tokens 1024-2047, wo, then batch 1.  The ACT queue carries
            # only the two tiny early tensors (its transfers hold the ACT
            # engine, which is harmless pre-attention).
            w_sb = {}
            wdefs = {"wk": wk, "wq": wq, "wv": wv}

            xTr = xT.rearrange("(k p) t -> p k t", p=128)
            xts = singles.tile([128, KCH, 1024], BF16, tag="xts")
            nc.sync.dma_start(out=xts, in_=xTr[:, :, 0:1024])
            for nm in ("wk", "wq", "wv"):
                t = singles.tile([128, KCH, DPC], BF16, tag=f"w{nm}",
                                 name=f"w{nm}")
                nc.sync.dma_start(
                    out=t, in_=wdefs[nm].rearrange("(k p) j -> p k j", p=128))
                w_sb[nm] = [t[:, k, :] for k in range(KCH)]
            xtm = singles.tile([128, KCH, 1024], BF16, tag="xtm")
            nc.sync.dma_start(out=xtm, in_=xTr[:, :, 1024:2048])
            wo_sb = singles.tile([DPC, C], BF16, tag="wo")
            nc.sync.dma_start(out=wo_sb, in_=wo)
            xtb = [singles.tile([128, 2048], BF16, tag=f"xtb{k}",
                                name=f"xtb{k}") for k in range(KCH)]
            for k in range(KCH):
                nc.sync.dma_start(
                    out=xtb[k], in_=xT[k * 128:(k + 1) * 128, 2048:4096])

            def xt_rhs(k, nch):
                if nch < 2:
                    return xts[:, k, nch * 512:nch * 512 + 512]
                if nch < 4:
                    off = (nch - 2) * 512
                    return xtm[:, k, off:off + 512]
                off = (nch - 4) * 512
                return xtb[k][:, off:off + 512]

            QT = singles.tile([128, T], BF16, tag="QT")
            KTt = singles.tile([128, T], BF16, tag="KT")
            VT = singles.tile([128, T], BF16, tag="VT")
            ctxTn = singles.tile([128, T], BF16, tag="ctxTn")
            vaug = [[singles.tile([128, KT16, D + 1], BF16,
                                  tag=f"vaug{b}{h}", name=f"vaug{b}{h}")
                     for h in range(HPC)] for b in range(B)]
            for b in range(B):
                for h in range(HPC):
                    nc.vector.memset(vaug[b][h], 1.0)


            with tc.tile_pool(name="psP", bufs=1, space="PSUM") as psP, \
                    tc.tile_pool(name="psO", bufs=1, space="PSUM") as psO, \
                    tc.tile_pool(name="psS", bufs=2, space="PSUM") as psS, \
                    tc.tile_pool(name="psC", bufs=1, space="PSUM") as psC, \
                    tc.tile_pool(name="esb", bufs=6) as esb, \
                    tc.tile_pool(name="nrm", bufs=4) as nrm, \
                    tc.tile_pool(name="csb", bufs=3) as csb, \
                    tc.tile_pool(name="osb", bufs=3) as osb:

                # REGULAR matmuls (not transposes: transpose-mode doesn't
                # count as PE-busy for HAM) to warm the clock gate to
                # K=8/8.  id_sb as both operands: no data deps beyond the
                # tiny iden DMA.  Interleaved between startup chain parts
                # (below) so the PE stays dense through the DMA-paced
                # region and the first chains run at full clock.
                warm_ctr = [0]

                def emit_warm(n):
                    # psS only: its 2 bufs alternate banks so junk MMs
                    # never serialize against the real psP/psO users.
                    for _ in range(n):
                        warm_ctr[0] += 1
                        ptw = psS.tile([128, 128], F32, tag="s",
                                       name="warm")
                        nc.tensor.matmul(out=ptw, lhsT=id_sb, rhs=id_sb,
                                         start=True, stop=True)

                # ---- projection chains, split into 2-matmul parts ----
                def chain_items(nm, dstT, nch, act_bias=False):
                    """Return 4 fill items (2 matmuls each); the last also
                    applies the bias.  V-transposes are separate items."""
                    ps_box = []

                    def part(p):
                        def emit():
                            if p == 0:
                                ps_box.append(
                                    psP.tile([128, 512], F32, tag="pj",
                                             name="pj"))
                            ps = ps_box[0]
                            for k in range(2 * p, 2 * p + 2):
                                nc.tensor.matmul(
                                    out=ps, lhsT=w_sb[nm][k],
                                    rhs=xt_rhs(k, nch),
                                    start=(k == 0), stop=(k == KCH - 1))
                            if p == 3:
                                dst = dstT[:, nch * 512:(nch + 1) * 512]
                                if act_bias:
                                    nc.scalar.activation(
                                        out=dst, in_=ps,
                                        func=mybir.ActivationFunctionType.Identity,
                                        bias=b_sb["b" + nm[1]], scale=1.0)
                                else:
                                    nc.vector.tensor_scalar_add(
                                        out=dst, in0=ps,
                                        scalar1=b_sb["b" + nm[1]])
                        return emit
                    return [part(p) for p in range(4)]

                def vtrans_items(b, nch):
                    """2 items, each transposing 2 of the 4 just-projected
                    128-token V tiles of (b, nch) into v_aug."""
                    def pair(j):
                        def emit():
                            for t16 in range(nch * 4 + 2 * j,
                                             nch * 4 + 2 * j + 2):
                                bt = t16 % KT16
                                pt = psO.tile([128, 128], BF16, tag="po",
                                              name="pt")
                                base = (b * N + bt * 128)
                                nc.tensor.transpose(
                                    pt, VT[:, base:base + 128], id_sb)
                                nc.vector.tensor_copy(
                                    out=vaug[b][0][:, bt, 0:D],
                                    in_=pt[:, 0:D])
                                nc.vector.tensor_copy(
                                    out=vaug[b][1][:, bt, 0:D],
                                    in_=pt[:, D:2 * D])
                        return emit
                    return [pair(0), pair(1)]

                def outproj_items(qch, last=False):
                    """8 items: one [128tok,512] out-proj matmul + copy
                    each; DMA after every 4 (one ot tile per nch2)."""
                    q0 = qch * 512
                    ot_box = {}

                    def item(j):
                        nch2, t4 = divmod(j, 4)

                        def emit():
                            if t4 == 0:
                                ot_box[nch2] = osb.tile(
                                    [128, 4, 512], BF16, tag="ot", name="ot")
                            ot = ot_box[nch2]
                            po = psO.tile([128, 512], F32, tag="po",
                                          name="po")
                            tok = q0 + t4 * 128
                            nc.tensor.matmul(
                                out=po, lhsT=ctxTn[:, tok:tok + 128],
                                rhs=wo_sb[:, nch2 * 512:(nch2 + 1) * 512],
                                start=True, stop=True)
                            nc.vector.tensor_copy(ot[:, t4, :], po)
                            if t4 == 3:
                                nc.sync.dma_start(
                                    out=out[q0:q0 + 512,
                                            nch2 * 512:(nch2 + 1) * 512]
                                    .rearrange("(t p) c -> p t c", p=128),
                                    in_=ot)
                        return emit
                    return [item(j) for j in range(8)]

                # ---- attention pieces ----
                def emit_scores(b, qch, kc):
                    # high_priority: at chunk boundaries the first scores
                    # matmul stalls ~1us on its psS bank; without the
                    # priority boost the scheduler slots fill matmuls
                    # between the two scores MMs of the pair, delaying the
                    # exp (the critical engine) by ~2us per boundary.
                    q0 = b * N + qch * 512
                    k0 = b * N + kc * 128
                    with tc.high_priority(offset=8):
                        pS = psS.tile([128, 1024], F32, tag="s", name="s")
                        for h in range(HPC):
                            nc.tensor.matmul(
                                out=pS[:, h * 512:(h + 1) * 512],
                                lhsT=KTt[h * D:(h + 1) * D, k0:k0 + 128],
                                rhs=QT[h * D:(h + 1) * D, q0:q0 + 512],
                                start=True, stop=True)
                        eS = esb.tile([128, 1024], BF16, tag="e", name="e")
                        nc.scalar.activation(
                            eS, pS, mybir.ActivationFunctionType.Exp)
                    return eS

                def emit_ctx(b, kc, eS, box):
                    for h in range(HPC):
                        nc.tensor.matmul(
                            out=box["t"][h], lhsT=vaug[b][h][:, kc, :],
                            rhs=eS[:, h * 512:(h + 1) * 512],
                            start=(kc == 0), stop=(kc == KT16 - 1))

                def emit_normalize(b, qch, ctx, last=False):
                    """ctx rows 0..63 / row 64 -> ctxTn (bf16).  Returns
                    (bcs, ctxss) when last=True for the per-qtile tail.
                    The cheap dn/recip ops go first on the DVE so both
                    gpsimd broadcasts start early; the heavy ctxs copies
                    overlap them."""
                    q0 = b * N + qch * 512
                    bcs, ctxss, rcs = [], [], []
                    for h in range(HPC):
                        dn = nrm.tile([1, 512], F32, tag=f"dn{h}",
                                      name=f"dn{h}")
                        nc.vector.tensor_copy(dn, ctx[h][D:D + 1, :])
                        rc = nrm.tile([1, 512], F32, tag=f"rc{h}",
                                      name=f"rc{h}")
                        nc.vector.reciprocal_approx_fast(rc, dn)
                        rcs.append(rc)
                    for h in range(HPC):
                        bc = nrm.tile([D, 512], F32, tag=f"bc{h}",
                                      name=f"bc{h}")
                        nc.gpsimd.partition_broadcast(bc, rcs[h])
                        bcs.append(bc)
                        ctxs = csb.tile([D, 512], F32, tag=f"ctxs{h}",
                                        name=f"ctxs{h}")
                        nc.vector.tensor_copy(ctxs, ctx[h][0:D, :])
                        ctxss.append(ctxs)
                    if last:
                        return bcs, ctxss
                    for h in range(HPC):
                        eng = nc.gpsimd if h == 0 else nc.vector
                        eng.tensor_mul(
                            out=ctxTn[h * D:(h + 1) * D, q0:q0 + 512],
                            in0=ctxss[h], in1=bcs[h])
                    return None

                # ---- chunk driver ----
                # chunk index ch = 0..7 -> (b, qch)
                def interleave(a, b):
                    o, i = [], 0
                    for x in a:
                        o.append(x)
                        if i < len(b):
                            o.append(b[i])
                            i += 1
                    o.extend(b[i:])
                    return o

                def fills_for(ch):
                    if ch == 0:
                        return []  # chunk 0 has an explicit slot plan
                    f = []
                    if ch <= 6:
                        # wq for the NEXT chunk, ACT bias: its write of QT
                        # must not queue behind the DVE backlog or the
                        # next chunk's first scores (and exp) stall.
                        f += chain_items("wq", QT, ch + 1, act_bias=True)
                    if ch in (1, 2, 3):
                        nch = ch + 3  # wk4/wv4 .. wk6/wv6
                        f += chain_items("wk", KTt, nch)
                        f += chain_items("wv", VT, nch)
                        f += vtrans_items(1, nch - 4)
                    if ch == 4:
                        f = (chain_items("wk", KTt, 7)
                             + chain_items("wv", VT, 7)
                             + vtrans_items(1, 3) + f)
                    # spread the out-proj items (each briefly serializes
                    # on the psO bank against its copy) between the chain
                    # items, and cap them at one per slot in the driver.
                    f = interleave([(False, x) for x in f],
                                   [(True, x) for x in outproj_items(ch - 1)])
                    return f

                # chunk 0 explicit slot plan: 3 items per kc slot
                c0_items = [(False, x) for x in (
                    chain_items("wv", VT, 0) + vtrans_items(0, 0)
                    + chain_items("wk", KTt, 1)
                    + chain_items("wv", VT, 1) + vtrans_items(0, 1)
                    + chain_items("wk", KTt, 2)
                    + chain_items("wv", VT, 2) + vtrans_items(0, 2)
                    + chain_items("wk", KTt, 3)
                    + chain_items("wv", VT, 3) + vtrans_items(0, 3)
                    + chain_items("wq", QT, 1, act_bias=True)
                )]

                # startup: only wk0 + wq0 before the exp stream begins;
                # warmup matmuls pad the DMA-paced stretches so HAM
                # reaches (and holds) K=8/8 before the chains run.
                emit_warm(8)
                for it in chain_items("wk", KTt, 0, act_bias=True):
                    emit_warm(4)
                    it()
                for it in chain_items("wq", QT, 0, act_bias=True):
                    emit_warm(2)
                    it()

                pend_ctx = []   # [(b, kc, eS, ctx)] lag-2 queue
                prev_norm = None

                for ch in range(8):
                    b, qch = divmod(ch, 4)
                    # psC tiles are allocated at kc==1, strictly AFTER the
                    # previous chunk's deferred ctx + normalize reads are
                    # emitted — allocating earlier would let this chunk's
                    # ctx matmuls race the previous normalize's PSUM reads.
                    box = {}
                    fills = c0_items if ch == 0 else fills_for(ch)
                    fi = 0
                    per_slot = 3 if ch == 0 else 2
                    for kc in range(KT16):
                        eS = emit_scores(b, qch, kc)
                        if pend_ctx and "t" in pend_ctx[0][3]:
                            emit_ctx(*pend_ctx.pop(0))
                        pend_ctx.append((b, kc, eS, box))
                        if kc == 1:
                            if prev_norm is not None:
                                # previous chunk's ctx is now fully emitted
                                prev_norm()
                                prev_norm = None
                            box["t"] = [
                                psC.tile([D + 1, 512], F32, tag=f"ctx{h}",
                                         name=f"ctx{h}")
                                for h in range(HPC)]
                        if prev_norm is not None:
                            # fills may include out-proj reads of ctxTn
                            # regions the pending normalize writes; popping
                            # them first would invert the dependency and
                            # read stale SBUF.
                            continue
                        took_op = False
                        for _ in range(per_slot):
                            if fi >= len(fills):
                                break
                            is_op, fn = fills[fi]
                            if is_op and took_op:
                                break  # max one psO out-proj item per slot
                            fn()
                            fi += 1
                            took_op = took_op or is_op
                    while fi < len(fills):
                        fills[fi][1]()
                        fi += 1

                    if ch < 7:
                        # leave the last 2 ctx for the next chunk's first
                        # slots (uniform lag across the boundary)
                        def mk_norm(b=b, qch=qch, ctx=box["t"]):
                            return lambda: emit_normalize(b, qch, ctx)
                        prev_norm = mk_norm()
                    else:
                        # tail: drain ctx, then normalize (cheap dn/recip
                        # ops first so the gpsimd broadcasts start early)
                        # and a per-qtile out-proj -> DMA pipeline.  Junk
                        # warm matmuls keep HAM at K=8/8 through the
                        # DVE/gpsimd-paced stretches.
                        while pend_ctx:
                            emit_ctx(*pend_ctx.pop(0))
                        emit_warm(4)
                        bcs, ctxss = emit_normalize(b, qch, box["t"],
                                                    last=True)
                        q0 = b * N + qch * 512
                        for t4 in range(4):
                            lo, hi = t4 * 128, (t4 + 1) * 128
                            for h in range(HPC):
                                eng = nc.gpsimd if h == 0 else nc.vector
                                eng.tensor_mul(
                                    out=ctxTn[h * D:(h + 1) * D,
                                              q0 + lo:q0 + hi],
                                    in0=ctxss[h][:, lo:hi],
                                    in1=bcs[h][:, lo:hi])
                            emit_warm(2)
                            # both 512-wide halves into one 2-bank psS
                            # tile: one CAST + one DMA per token tile
                            # instead of two of each.
                            tok = q0 + t4 * 128
                            po = psS.tile([128, 1024], F32, tag="s",
                                          name="pol")
                            for nch2 in range(2):
                                nc.tensor.matmul(
                                    out=po[:, nch2 * 512:(nch2 + 1) * 512],
                                    lhsT=ctxTn[:, tok:tok + 128],
                                    rhs=wo_sb[:, nch2 * 512:(nch2 + 1) * 512],
                                    start=True, stop=True)
                            otl = osb.tile([128, 1024], BF16, tag="otl",
                                           name="otl", bufs=3)
                            nc.vector.tensor_copy(otl, po)
                            nc.sync.dma_start(
                                out=out[tok:tok + 128, :], in_=otl)
    return nc


_NC_CACHE = None


def _get_nc():
    global _NC_CACHE
    if _NC_CACHE is None:
        nc = bacc.Bacc("TRN2", target_bir_lowering=False)
        build_core_program(nc)
        nc.finalize()
        _NC_CACHE = nc
    return _NC_CACHE


def make_in_maps(x, Wq, bq, Wk, bk, Wv, bv, Wo):
    bf = ml_dtypes.bfloat16
    x = np.asarray(x, np.float32).reshape(T, C)
    xT_bf = np.ascontiguousarray(x.T).astype(bf)
    iden = np.eye(128, dtype=bf)
    Wq = np.asarray(Wq, np.float32)
    Wk = np.asarray(Wk, np.float32)
    Wv = np.asarray(Wv, np.float32)
    Wo = np.asarray(Wo, np.float32)
    bq = np.asarray(bq, np.float32)
    bk = np.asarray(bk, np.float32)
    bv = np.asarray(bv, np.float32)
    in_maps = []
    for cidx in range(8):
        hs = slice(cidx * DPC, (cidx + 1) * DPC)
        in_maps.append(dict(
            xT=xT_bf,
            wq=np.ascontiguousarray(Wq[:, hs] * 0.125).astype(bf),
            wk=np.ascontiguousarray(Wk[:, hs]).astype(bf),
            wv=np.ascontiguousarray(Wv[:, hs]).astype(bf),
            wo=np.ascontiguousarray(Wo[hs, :]).astype(bf),
            bqkv=np.stack([bq[hs] * 0.125, bk[hs], bv[hs]],
                          axis=1).astype(np.float32),
            iden=iden,
        ))
    return in_maps


def kernel(x, Wq, bq, Wk, bk, Wv, bv, Wo, bo, _trace=False, _trace_kwargs=None):
    in_maps = make_in_maps(x, Wq, bq, Wk, bk, Wv, bv, Wo)
    nc = _get_nc()
    res = run_bass_kernel_spmd(
        nc, in_maps, core_ids=list(range(8)),
        trace=_trace, **(_trace_kwargs or {}))
    acc = res.results[0]["out"].astype(np.float32)
    for cidx in range(1, 8):
        acc += res.results[cidx]["out"].astype(np.float32)
    acc += np.asarray(bo, np.float32)[None, :]
    out = acc.reshape(B, N, C)
    kernel.last_results = res
    return out
